# revision 2
# baseline (speedup 1.0000x reference)
"""Trainium2 Bass kernel for GatbertEmbeddings (segment_reduce).

Computes, for full inputs:
    table = emb_table with row 0 zeroed (padding_idx=0)
    sub_emb = table[subword_ids]                         # [B, S, H]
    pooled[b, n, :] = sum over nnz entries e with mask_batch[e]==b,
        mask_node[e]==n of mask_values[e] * sub_emb[b, mask_sub[e], :]
    out = LayerNorm(pooled) * gamma + beta               # [B, MAX_NODES, H]

Strategy: data-parallel over batch across 8 NeuronCores (4 batches/core),
embedding table replicated. Per core, per batch:
  - indirect-DMA gather of the 512 token rows from the table (dma_gather)
  - the sparse mask is shipped as a dense per-batch matrix A^T [S, NODES]
    (built host-side from the COO entries during sharding); the weighted
    segment-sum is then pooled = A @ E as TensorEngine matmuls
  - LayerNorm via bn_stats/bn_aggr + fused scale/bias activation
"""

import numpy as np

import concourse.bass as bass
import concourse.bacc as bacc
import concourse.tile as tile
import concourse.mybir as mybir
from concourse.bass_utils import run_bass_kernel_spmd

B, S, NNZ = 32, 512, 16384
V, H, NODES = 30522, 768, 256
NCORES = 8
BLOC = B // NCORES          # batches per core
EPS = 1e-12
KC = S // 128               # K chunks per batch (contraction over seq pos)
MT = NODES // 128            # M tiles (node dim)
NSPLIT = (0, 512, 768)       # PSUM free-dim split (bank-aligned, <=512 per matmul)

# Compute dtype for the gather + matmul operands. float32 is exact;
# float32r / bfloat16 are faster alternatives (see _build callers).
_CACHE = {}


def _build(dt_name: str, apply_gamma_beta: bool):
    key = (dt_name, apply_gamma_beta)
    if key in _CACHE:
        return _CACHE[key]
    DT = getattr(mybir.dt, dt_name)
    nc = bacc.Bacc("TRN2", target_bir_lowering=False, debug=False,
                   num_devices=NCORES)
    tok = nc.dram_tensor("tok", [BLOC, 128, S // 16], mybir.dt.int16,
                         kind="ExternalInput")
    table = nc.dram_tensor("table", [V, H], DT, kind="ExternalInput")
    amat = nc.dram_tensor("amat", [BLOC, 128, KC, NODES], DT,
                          kind="ExternalInput")
    gamma = nc.dram_tensor("gamma", [1, H], mybir.dt.float32,
                           kind="ExternalInput")
    beta = nc.dram_tensor("beta", [1, H], mybir.dt.float32,
                          kind="ExternalInput")
    out = nc.dram_tensor("out", [BLOC, NODES, H], mybir.dt.float32,
                         kind="ExternalOutput")

    with tile.TileContext(nc) as tc:
        with (
            tc.tile_pool(name="singles", bufs=1) as singles,
            tc.tile_pool(name="idxp", bufs=2) as idxp,
            tc.tile_pool(name="ep", bufs=2) as ep,
            tc.tile_pool(name="apool", bufs=2) as apool,
            tc.tile_pool(name="psp", bufs=2, space="PSUM") as psp,
            tc.tile_pool(name="statp", bufs=6) as statp,
            tc.tile_pool(name="obp", bufs=3) as obp,
        ):
            eps_t = singles.tile([128, 1], mybir.dt.float32)
            nc.vector.memset(eps_t, EPS)
            if apply_gamma_beta:
                gamma_t = singles.tile([128, H], mybir.dt.float32)
                beta_t = singles.tile([128, H], mybir.dt.float32)
                gamma_b = bass.AP(tensor=gamma.tensor, offset=0,
                                  ap=[[0, 128], [1, H]])
                beta_b = bass.AP(tensor=beta.tensor, offset=0,
                                 ap=[[0, 128], [1, H]])
                nc.sync.dma_start(out=gamma_t[:], in_=gamma_b)
                nc.sync.dma_start(out=beta_t[:], in_=beta_b)

            for b in range(BLOC):
                idx_t = idxp.tile([128, S // 16], mybir.dt.int16)
                nc.sync.dma_start(out=idx_t[:], in_=tok[b])
                e_t = ep.tile([128, KC, H], DT)
                nc.gpsimd.dma_gather(e_t[:], table[:], idx_t[:], S, S, H)
                a_t = apool.tile([128, KC, NODES], DT)
                nc.sync.dma_start(out=a_t[:], in_=amat[b])

                for m in range(MT):
                    ps = psp.tile([128, H], mybir.dt.float32)
                    for ni in range(len(NSPLIT) - 1):
                        n0, n1 = NSPLIT[ni], NSPLIT[ni + 1]
                        for c in range(KC):
                            nc.tensor.matmul(
                                ps[:, n0:n1],
                                a_t[:, c, m * 128:(m + 1) * 128],
                                e_t[:, c, n0:n1],
                                start=(c == 0),
                                stop=(c == KC - 1),
                            )
                    # LayerNorm over the free (hidden) dim of ps [128, H]
                    stats = statp.tile([128, 3, 6], mybir.dt.float32)
                    for j in range(3):
                        nc.vector.bn_stats(out=stats[:, j, :],
                                           in_=ps[:, j * 256:(j + 1) * 256])
                    mv = statp.tile([128, 2], mybir.dt.float32)
                    nc.vector.bn_aggr(out=mv[:], in_=stats[:])
                    rstd = statp.tile([128, 1], mybir.dt.float32)
                    nc.scalar.activation(out=rstd[:], in_=mv[:, 1:2],
                                         func=mybir.ActivationFunctionType.Sqrt,
                                         bias=eps_t[:], scale=1.0)
                    nc.vector.reciprocal(out=rstd[:], in_=rstd[:])
                    nmr = statp.tile([128, 1], mybir.dt.float32)
                    # nmr = -mu * rstd
                    nc.vector.tensor_scalar(out=nmr[:], in0=mv[:, 0:1],
                                            scalar1=rstd[:], scalar2=-1.0,
                                            op0=mybir.AluOpType.mult,
                                            op1=mybir.AluOpType.mult)
                    osb = obp.tile([128, H], mybir.dt.float32)
                    # osb = ps * rstd - mu * rstd
                    nc.scalar.activation(out=osb[:], in_=ps[:],
                                         func=mybir.ActivationFunctionType.Identity,
                                         bias=nmr[:], scale=rstd[:])
                    if apply_gamma_beta:
                        nc.vector.tensor_mul(osb[:], osb[:], gamma_t[:])
                        nc.vector.tensor_add(osb[:], osb[:], beta_t[:])
                    nc.sync.dma_start(out=out[b, m * 128:(m + 1) * 128, :],
                                      in_=osb[:])
    nc.compile()
    _CACHE[key] = nc
    return nc


def _prep_inputs(subword_ids, mask_batch, mask_node, mask_sub, mask_values,
                 emb_table, gamma, beta, np_dt):
    """Shard inputs: batches 4i..4i+3 -> core i; table replicated."""
    subword_ids = np.asarray(subword_ids)
    mask_batch = np.asarray(mask_batch).astype(np.int64)
    mask_node = np.asarray(mask_node).astype(np.int64)
    mask_sub = np.asarray(mask_sub).astype(np.int64)
    mask_values = np.asarray(mask_values).astype(np.float32)
    emb_table = np.asarray(emb_table).astype(np.float32)
    gamma = np.asarray(gamma).astype(np.float32).reshape(1, H)
    beta = np.asarray(beta).astype(np.float32).reshape(1, H)

    table = emb_table.copy()
    table[0, :] = 0.0  # padding_idx
    table = table.astype(np_dt)

    # Dense per-batch mask A^T[b][s, node] = sum of values (duplicates add)
    a_full = np.zeros((B, S, NODES), dtype=np.float32)
    np.add.at(a_full, (mask_batch, mask_sub, mask_node), mask_values)

    in_maps = []
    for i in range(NCORES):
        sl = slice(BLOC * i, BLOC * (i + 1))
        toks = subword_ids[sl].astype(np.int64)  # [BLOC, S]
        # dma_gather index layout: idx j at [j % 16, j // 16], replicated
        # across the 8 Q7 16-partition groups.
        wrapped = toks.reshape(BLOC, S // 16, 16).transpose(0, 2, 1)  # [BLOC,16,S//16]
        wrapped = np.tile(wrapped, (1, 8, 1)).astype(np.int16)        # [BLOC,128,S//16]
        # A^T reshaped so SBUF partition p holds [KC, NODES] slabs:
        # a[b, p, c, node] = A^T[b, c*128+p, node]
        a_core = (a_full[sl]                      # [BLOC, S, NODES]
                  .reshape(BLOC, KC, 128, NODES)
                  .transpose(0, 2, 1, 3)          # [BLOC, 128, KC, NODES]
                  .astype(np_dt))
        in_maps.append({
            "tok": np.ascontiguousarray(wrapped),
            "table": table,
            "amat": np.ascontiguousarray(a_core),
            "gamma": gamma,
            "beta": beta,
        })
    return in_maps


def kernel(subword_ids, mask_batch, mask_node, mask_sub, mask_values,
           emb_table, gamma, beta):
    dt_name = "float32"
    np_dt = np.float32
    g = np.asarray(gamma).astype(np.float32)
    bt = np.asarray(beta).astype(np.float32)
    apply_gb = not (np.all(g == 1.0) and np.all(bt == 0.0))

    nc = _build(dt_name, apply_gb)
    in_maps = _prep_inputs(subword_ids, mask_batch, mask_node, mask_sub,
                           mask_values, emb_table, gamma, beta, np_dt)
    res = run_bass_kernel_spmd(nc, in_maps, list(range(NCORES)))
    outs = [res.results[i]["out"] for i in range(NCORES)]
    return np.concatenate(outs, axis=0).astype(np.float32)
